# revision 5
# baseline (speedup 1.0000x reference)
"""4-D average pool (kernel=2, stride=2) over [2,16,32,32,32,32] f32, on 8 NeuronCores.

Strategy: data-parallel over the 32 (b,c) slices -> 4 slices per core; the
per-core input is a contiguous [4096, 1024] f32 block (rows = (slice,d1,d2),
cols = (d3,d4)).

Fully-contiguous loads on the SP HWDGE ring (2 MiB bulk, tapering to 512 KiB
at the end), whole 16 MiB shard SBUF-resident so load DMAs carry no waits.
Compute per 128-row chunk:
  - two DVE adds pool the free dim (d4 pairs, then d3 pairs); the second add
    writes bf16 (rel err ~2^-9, far under the 2e-2 gate)
  - one bf16 matmul per 256-row block with a constant [128, 32] pooling
    matrix pools the (d1,d2) partition pairs; bf16 is single-pass on the PE
    (fp32 LOW/HI needs two) and the 1/16 scale is exact in bf16
  - matmul pairs share a 2-bank PSUM tile; ONE ScalarE copy moves [32, 1024]
    per pair to a per-1024-row [32, 2048] SBUF output tile
  - 4-6 batched stores (ACT ring) instead of 16 small ones
The instruction stream is shaped so nearly every instruction carries at most
ONE semaphore wait (slot-reuse releases land on the same engine in program
order): bacc then emits almost no event semaphores, which collapses the
kernel-tail sem-reset chain + barrier (the measured window includes it) from
~8.7us to ~2.5us.  The final 512 input rows are processed in 256/128/128-row
blocks with their own small stores so the last-load -> last-store dependency
chain stays short.
"""

import sys

import ml_dtypes
import numpy as np

if "/opt/trn_rl_repo" not in sys.path:
    sys.path.insert(0, "/opt/trn_rl_repo")

import concourse.bacc as bacc
import concourse.bass as bass
import concourse.tile as tile
from concourse import mybir
from concourse.bass_utils import run_bass_kernel_spmd

N_CORES = 8
SLICES_PER_CORE = 4  # 32 (b,c) slices / 8 cores
ROWS = SLICES_PER_CORE * 1024  # 4096
# DMA schedule (start_row, n_rows): 2 MiB loads for the bulk, tapering to
# 512 KiB at the end to shorten the tail chain.
LOADS = [(r, 512) for r in range(0, 3584, 512)] + [(3584, 256), (3840, 128), (3968, 128)]
F32 = mybir.dt.float32
BF16 = mybir.dt.bfloat16


def _build_pm() -> np.ndarray:
    # B[r, j] = 1/16 iff chunk row r = 32*d1l + d2 pools into chunk output
    # row j = 16*(d1l//2) + d2//2   (d1l in [0,4), d2 in [0,32))
    b = np.zeros((128, 32), np.float32)
    for d1l in range(4):
        for d2 in range(32):
            b[32 * d1l + d2, 16 * (d1l // 2) + d2 // 2] = 1.0 / 16.0
    return b.astype(ml_dtypes.bfloat16)


def build_nc() -> bass.Bass:
    # Bacc (not raw Bass): its compile() splits multi-sem sync waits into
    # event-semaphore instructions (TRN2 allows one wait per instruction).
    nc = bacc.Bacc()
    x = nc.dram_tensor("x", [ROWS, 1024], F32, kind="ExternalInput")
    pm = nc.dram_tensor("pm", [128, 32], BF16, kind="ExternalInput")
    y = nc.dram_tensor("y", [ROWS // 4, 256], F32, kind="ExternalOutput")

    n2m = sum(1 for _, n in LOADS if n == 512)
    n1m = sum(1 for _, n in LOADS if n == 256)
    nhm = sum(1 for _, n in LOADS if n == 128)

    with tile.TileContext(nc) as tc:
        with (
            tc.tile_pool(name="pmp", bufs=1) as pmp,
            # one pool per load size, bufs = count -> no slot reuse; the
            # whole 16 MiB input is SBUF-resident
            tc.tile_pool(name="in2m", bufs=max(n2m, 1)) as in2m,
            tc.tile_pool(name="in1m", bufs=max(n1m, 1)) as in1m,
            tc.tile_pool(name="inhm", bufs=max(nhm, 1)) as inhm,
            tc.tile_pool(name="m1p", bufs=2) as m1p,
            # m2/ob: one slot per tile (no reuse -> consumers never carry a
            # second "slot released" sem wait); psum capped by the 8 banks,
            # reuse waits there are absorbed into LDWEIGHTS by bacc
            tc.tile_pool(name="m2p", bufs=17) as m2p,
            tc.tile_pool(name="psp", bufs=4, space=bass.MemorySpace.PSUM) as psp,
            tc.tile_pool(name="obp", bufs=4) as obp,
        ):
            pm_t = pmp.tile([128, 32], BF16)

            # ---- loads (SP ring, no waits) --------------------------------
            tiles = {}
            for li, (row, nrows) in enumerate(LOADS):
                nqt = nrows // 128
                pool = {512: in2m, 256: in1m, 128: inhm}[nrows]
                t = pool.tile([128, 1024 * nqt], F32, tag="t")
                src = x[row : row + nrows, :].rearrange("(q p) c -> p q c", p=128)
                nc.sync.dma_start(t[:].rearrange("p (q c) -> p q c", q=nqt), src)
                tiles[row] = (t, nqt)
                if li == 0:
                    # pm load after the first bulk DMA: it is only needed by
                    # the first matmul, not on the critical path
                    nc.sync.dma_start(pm_t[:], pm[:])

            # ---- compute ---------------------------------------------------
            def dve_block(tv, nq, m2half):
                """Pool d4 then d3 pairs of a [128, 1024*nq] view; bf16 out."""
                v = tv.rearrange("p (q d3 o4 e4) -> p q d3 o4 e4", q=nq, d3=32, o4=16)
                m1 = m1p.tile([128, 512 * nq], F32, tag="m1")
                m1v = m1[:].rearrange("p (q d3 o4) -> p q d3 o4", q=nq, d3=32)
                nc.vector.tensor_add(m1v, v[:, :, :, :, 0], v[:, :, :, :, 1])

                w = m1[:].rearrange("p (q o3 e3 o4) -> p q o3 e3 o4", q=nq, o3=16, o4=16)
                m2v = m2half.rearrange("p (q o3 o4) -> p q o3 o4", q=nq, o3=16)
                nc.vector.tensor_add(m2v, w[:, :, :, 0, :], w[:, :, :, 1, :])

            # Each "unit" gets one PSUM tile, 1-2 matmuls, one ScalarE copy.
            # groups: list of units, each a list of (row, nq) 128*nq-row
            # blocks; plus the store splits (ob column ranges).
            # g0..g2: 1024 rows each, two 2-block units, one 256 KiB store.
            # g3: tapered units (256/128/128-row blocks get their own psum +
            # copy) and a 192/64 KiB store split -> short last-load chain.
            groups = [
                ([[(0, 2), (256, 2)], [(512, 2), (768, 2)]], [(0, 2048)]),
                ([[(1024, 2), (1280, 2)], [(1536, 2), (1792, 2)]], [(0, 2048)]),
                ([[(2048, 2), (2304, 2)], [(2560, 2), (2816, 2)]], [(0, 2048)]),
                (
                    [[(3072, 2), (3328, 2)], [(3584, 2)], [(3840, 1)], [(3968, 1)]],
                    [(0, 1536), (1536, 512)],
                ),
            ]

            def tile_view(row, nq):
                # locate the loaded tile holding rows [row, row+128*nq)
                for lrow, lnr in LOADS:
                    if lrow <= row < lrow + lnr:
                        t, nqt = tiles[lrow]
                        q0 = (row - lrow) // 128
                        return t[:, 1024 * q0 : 1024 * (q0 + nq)]
                raise AssertionError(row)

            for gi, (units, store_splits) in enumerate(groups):
                ob = obp.tile([32, 2048], F32, tag="ob")
                oc = 0  # ob column cursor (f32 elems)
                for unit in units:
                    width = sum(256 * nq for _, nq in unit)
                    ps = psp.tile([32, width], F32, tag="ps")
                    used = 0
                    for row, nq in unit:
                        dve_m2 = m2p.tile([128, 256 * nq], BF16, tag="m2")
                        dve_block(tile_view(row, nq), nq, dve_m2[:])
                        nc.tensor.matmul(
                            ps[:, used : used + 256 * nq],
                            pm_t[:],
                            dve_m2[:],
                            start=True,
                            stop=True,
                        )
                        used += 256 * nq
                    nc.scalar.copy(ob[:, oc : oc + width], ps[:])
                    oc += width
                orow = 256 * gi
                for col0, ncols in store_splits:
                    g = ncols // 256
                    r0 = orow + col0 // 256 * 32
                    dst = y[r0 : r0 + 32 * g, :].rearrange("(g j) c -> j g c", j=32)
                    nc.scalar.dma_start(
                        dst, ob[:, col0 : col0 + ncols].rearrange("j (g c) -> j g c", g=g)
                    )

    nc.compile()
    return nc


_NC_CACHE: bass.Bass | None = None


def kernel(nd_tensor: np.ndarray, _trace: bool = False):
    global _NC_CACHE
    x = np.ascontiguousarray(np.asarray(nd_tensor, dtype=np.float32)).reshape(
        32, 1024, 1024
    )
    pm = _build_pm()
    if _NC_CACHE is None:
        _NC_CACHE = build_nc()
    nc = _NC_CACHE

    in_maps = [
        {
            "x": np.ascontiguousarray(
                x[SLICES_PER_CORE * i : SLICES_PER_CORE * (i + 1)]
            ).reshape(ROWS, 1024),
            "pm": pm,
        }
        for i in range(N_CORES)
    ]
    res = run_bass_kernel_spmd(
        nc, in_maps, core_ids=list(range(N_CORES)), trace=_trace
    )
    out = np.stack([res.results[i]["y"] for i in range(N_CORES)])  # [8,1024,256]
    out = out.reshape(2, 16, 16, 16, 16, 16).astype(np.float32)
    if _trace:
        kernel.last_results = res
    return out


# revision 19
# speedup vs baseline: 1.1005x; 1.1005x over previous
"""4-D average pool (kernel=2, stride=2) over [2,16,32,32,32,32] f32, on 8 NeuronCores.

Strategy: data-parallel over the 32 (b,c) slices -> 4 slices per core; the
per-core input is a contiguous [4096, 1024] f32 block (rows = (slice,d1,d2),
cols = (d3,d4)).

Loads on the SP HWDGE ring with 4 KiB descriptor elements (measured faster
than 16 KiB: better HBM bank-level parallelism), whole 16 MiB shard
SBUF-resident so load DMAs carry no waits.  Load sizes taper (2 MiB bulk ->
1 MiB -> split 256 KiB column-halves) so DVE work arrives in small bursts
and there is no compute backlog when the last byte lands.
Compute per 128-row chunk:
  - two DVE adds pool the free dim (d4 pairs fp32->bf16, then d3 pairs in
    bf16 -- contiguous bf16 operands engage the DVE 2x packed mode); total
    bf16 rounding error ~3e-3 rel, far under the 2e-2 gate
  - one bf16 matmul per 256-row block with a constant [128, 32] pooling
    matrix pools the (d1,d2) partition pairs; bf16 is single-pass on the PE
    (fp32 LOW/HI needs two) and the 1/16 scale is exact in bf16
  - matmul pairs share a 2-bank PSUM tile; ONE ScalarE copy moves [32, 1024]
    per pair to a per-1024-row [32, 2048] SBUF output tile
  - batched 256 KiB stores on the ACT ring for the first 3 groups (their
    slow 1 KiB-descriptor drain hides under the load stream); the tail
    group's stores go on the by-then-idle SP ring so the final PSUM copies
    never queue behind store triggers on the ACT sequencer
Tail: the last two 128-row blocks are processed as column-halves pipelined
with their split loads; the 3840 block runs on the otherwise-idle GpSimd so
the DVE is free the moment the final bytes land.  Nearly every instruction
carries at most ONE semaphore wait (slot-reuse releases land on the same
engine in program order), minimizing bacc event semaphores.
Measured (8-core, profiled): ~59.1-60.5 us typical, vs 62.5 us baseline;
the remaining budget is ~2.5 us head, ~44 us HBM-bound load stream at
~383 GB/s/core, ~3.5 us tail chain, and a fixed ~8.5 us NEFF-wrapper
teardown (sem resets + barriers) that every kernel in this harness pays.
"""

import sys

import ml_dtypes
import numpy as np

if "/opt/trn_rl_repo" not in sys.path:
    sys.path.insert(0, "/opt/trn_rl_repo")

import concourse.bacc as bacc
import concourse.bass as bass
import concourse.tile as tile
from concourse import mybir
from concourse.bass_utils import run_bass_kernel_spmd

N_CORES = 8
SLICES_PER_CORE = 4  # 32 (b,c) slices / 8 cores
ROWS = SLICES_PER_CORE * 1024  # 4096
# DMA schedule (start_row, n_rows): small first load (cheap trigger -> data
# starts sooner), 2 MiB bulk, then 1 MiB loads near the end so DVE work
# arrives in small bursts (no backlog at the last byte), 512 KiB split-in-
# half tail loads.
LOADS = (
    [(0, 256)]
    + [(r, 512) for r in range(256, 2816, 512)]
    + [(r, 256) for r in range(2816, 3840, 256)]
    + [(3840, 128), (3968, 128)]
)
F32 = mybir.dt.float32
BF16 = mybir.dt.bfloat16


def _build_pm() -> np.ndarray:
    # B[r, j] = 1/16 iff chunk row r = 32*d1l + d2 pools into chunk output
    # row j = 16*(d1l//2) + d2//2   (d1l in [0,4), d2 in [0,32))
    b = np.zeros((128, 32), np.float32)
    for d1l in range(4):
        for d2 in range(32):
            b[32 * d1l + d2, 16 * (d1l // 2) + d2 // 2] = 1.0 / 16.0
    return b.astype(ml_dtypes.bfloat16)


def build_nc() -> bass.Bass:
    # Bacc (not raw Bass): its compile() splits multi-sem sync waits into
    # event-semaphore instructions (TRN2 allows one wait per instruction).
    nc = bacc.Bacc()
    x = nc.dram_tensor("x", [ROWS, 1024], F32, kind="ExternalInput")
    pm = nc.dram_tensor("pm", [128, 32], BF16, kind="ExternalInput")
    y = nc.dram_tensor("y", [ROWS // 4, 256], F32, kind="ExternalOutput")

    n2m = sum(1 for _, n in LOADS if n == 512)
    n1m = sum(1 for _, n in LOADS if n == 256)
    nhm = sum(1 for _, n in LOADS if n == 128)

    with tile.TileContext(nc) as tc:
        with (
            tc.tile_pool(name="pmp", bufs=1) as pmp,
            # one pool per load size, bufs = count -> no slot reuse; the
            # whole 16 MiB input is SBUF-resident
            tc.tile_pool(name="in2m", bufs=max(n2m, 1)) as in2m,
            tc.tile_pool(name="in1m", bufs=max(n1m, 1)) as in1m,
            tc.tile_pool(name="inhm", bufs=max(nhm, 1)) as inhm,
            tc.tile_pool(name="m1p", bufs=2) as m1p,
            # m2/ob: one slot per tile (no reuse -> consumers never carry a
            # second "slot released" sem wait); psum capped by the 8 banks,
            # reuse waits there are absorbed into LDWEIGHTS by bacc
            tc.tile_pool(name="m2p", bufs=19) as m2p,
            tc.tile_pool(name="psp", bufs=3, space=bass.MemorySpace.PSUM) as psp,
            # dedicated PSUM slots for the two final 128-row units so their
            # matmuls never wait on a big-unit copy to release a bank
            tc.tile_pool(name="pst", bufs=2, space=bass.MemorySpace.PSUM) as pst,
            tc.tile_pool(name="obp", bufs=4) as obp,
        ):
            pm_t = pmp.tile([128, 32], BF16)

            # ---- loads (SP ring, no waits) --------------------------------
            tiles = {}
            for li, (row, nrows) in enumerate(LOADS):
                nqt = nrows // 128
                pool = {512: in2m, 256: in1m, 128: inhm}[nrows]
                t = pool.tile([128, 1024 * nqt], F32, tag="t")
                if li >= len(LOADS) - 2:
                    # split the last two 128-row loads into column-halves so
                    # compute on each half overlaps the next half's DMA and
                    # the last-byte -> last-store chain is half as deep
                    for h in range(2):
                        nc.sync.dma_start(
                            t[:, 512 * h : 512 * (h + 1)],
                            x[row : row + 128, 512 * h : 512 * (h + 1)],
                        )
                else:
                    src = x[row : row + nrows, :].rearrange("(q p) c -> p q c", p=128)
                    nc.sync.dma_start(t[:].rearrange("p (q c) -> p q c", q=nqt), src)
                tiles[row] = (t, nqt)
                if li == 0:
                    # pm load after the first bulk DMA: it is only needed by
                    # the first matmul, not on the critical path
                    nc.sync.dma_start(pm_t[:], pm[:])

            # ---- compute ---------------------------------------------------
            # s1 writes bf16: halves m1 footprint and makes s2 an all-bf16
            # tensor_tensor (eligible for the DVE 2x packed mode); error from
            # bf16 rounding of 2- and 4-element sums is ~2^-9 rel, far under
            # the 2e-2 gate.
            def dve_block(tv, nq, m2half):
                """Pool d4 then d3 pairs of a [128, 1024*nq] view; bf16 out."""
                v = tv.rearrange("p (q d3 o4 e4) -> p q d3 o4 e4", q=nq, d3=32, o4=16)
                m1 = m1p.tile([128, 512 * nq], BF16, tag="m1")
                m1v = m1[:].rearrange("p (q d3 o4) -> p q d3 o4", q=nq, d3=32)
                nc.vector.tensor_add(m1v, v[:, :, :, :, 0], v[:, :, :, :, 1])

                w = m1[:].rearrange("p (q o3 e3 o4) -> p q o3 e3 o4", q=nq, o3=16, o4=16)
                m2v = m2half.rearrange("p (q o3 o4) -> p q o3 o4", q=nq, o3=16)
                nc.vector.tensor_add(m2v, w[:, :, :, 0, :], w[:, :, :, 1, :])

            # Each "unit" gets one PSUM tile, 1-2 matmuls, one ScalarE copy.
            # groups: list of units, each a list of (row, nq) 128*nq-row
            # blocks; plus the store splits (ob column ranges).
            # g0..g2: 1024 rows each, two 2-block units, one 256 KiB store.
            # g3: tapered units (256/128/128-row blocks get their own psum +
            # copy) and a 192/64 KiB store split -> short last-load chain.
            groups = [
                ([[(0, 2), (256, 2)], [(512, 2), (768, 2)]], [(0, 2048)]),
                ([[(1024, 2), (1280, 2)], [(1536, 2), (1792, 2)]], [(0, 2048)]),
                ([[(2048, 2), (2304, 2)], [(2560, 2), (2816, 2)]], [(0, 2048)]),
                (
                    [[(3072, 2), (3328, 2)], [(3584, 2)], [(3840, 1)], [(3968, 1)]],
                    [(0, 1024), (1024, 512), (1536, 256), (1792, 256)],
                ),
            ]

            def tile_view(row, nq):
                # locate the loaded tile holding rows [row, row+128*nq)
                for lrow, lnr in LOADS:
                    if lrow <= row < lrow + lnr:
                        t, nqt = tiles[lrow]
                        q0 = (row - lrow) // 128
                        return t[:, 1024 * q0 : 1024 * (q0 + nq)]
                raise AssertionError(row)

            def emit_store(y, ob, orow, col0, ncols, engine):
                g = ncols // 256
                r0 = orow + col0 // 256 * 32
                dst = y[r0 : r0 + 32 * g, :].rearrange("(g j) c -> j g c", j=32)
                engine.dma_start(
                    dst, ob[:, col0 : col0 + ncols].rearrange("j (g c) -> j g c", g=g)
                )

            for gi, (units, store_splits) in enumerate(groups):
                ob = obp.tile([32, 2048], F32, tag="ob")
                oc = 0  # ob column cursor (f32 elems)
                splits = list(store_splits)
                orow = 256 * gi
                # g3 tail stores go on the (idle-by-then) SP ring so the
                # final PSUM->SBUF copies never queue behind store triggers
                # on the ACT sequencer
                store_engine = nc.sync if gi == 3 else nc.scalar
                for unit in units:
                    width = sum(256 * nq for _, nq in unit)
                    halved = unit[0][0] >= 3840
                    pool = pst if halved else psp
                    ps = pool.tile([32, width], F32, tag="pst" if halved else "ps")
                    used = 0
                    for row, nq in unit:
                        if halved:
                            # final 128-row blocks: column-halves pipelined
                            # with their split loads; each half gets its own
                            # small matmul so the chain off the last DMA is
                            # short
                            t, _ = tiles[row]
                            # the 3840 block runs on GpSimd so the DVE is
                            # already idle when the very last (3968) halves
                            # land; the final chain starts immediately
                            veng = nc.gpsimd if row == 3840 else nc.vector
                            for h in range(2):
                                m1 = m1p.tile([128, 256], BF16, tag="m1g" if row == 3840 else "m1")
                                v = t[:, 512 * h : 512 * (h + 1)].rearrange(
                                    "p (d3 o4 e4) -> p d3 o4 e4", d3=16, o4=16
                                )
                                m1v = m1[:].rearrange("p (d3 o4) -> p d3 o4", d3=16)
                                veng.tensor_add(
                                    m1v, v[:, :, :, 0], v[:, :, :, 1]
                                )
                                m2h = m2p.tile([128, 128], BF16, tag="m2")
                                w = m1[:].rearrange(
                                    "p (o3 e3 o4) -> p o3 e3 o4", o3=8, o4=16
                                )
                                m2v = m2h[:].rearrange("p (o3 o4) -> p o3 o4", o3=8)
                                veng.tensor_add(
                                    m2v, w[:, :, 0, :], w[:, :, 1, :]
                                )
                                nc.tensor.matmul(
                                    ps[:, 128 * h : 128 * (h + 1)],
                                    pm_t[:],
                                    m2h[:],
                                    start=True,
                                    stop=True,
                                )
                            used += 256
                        else:
                            dve_m2 = m2p.tile([128, 256 * nq], BF16, tag="m2")
                            dve_block(tile_view(row, nq), nq, dve_m2[:])
                            nc.tensor.matmul(
                                ps[:, used : used + 256 * nq],
                                pm_t[:],
                                dve_m2[:],
                                start=True,
                                stop=True,
                            )
                            used += 256 * nq
                    nc.scalar.copy(ob[:, oc : oc + width], ps[:])
                    oc += width
                    # emit any store split fully covered by copies so far
                    while splits and splits[0][0] + splits[0][1] <= oc:
                        col0, ncols = splits.pop(0)
                        emit_store(y, ob, orow, col0, ncols, store_engine)
                for col0, ncols in splits:
                    emit_store(y, ob, orow, col0, ncols, store_engine)

    nc.compile()
    return nc


_NC_CACHE: bass.Bass | None = None


def kernel(nd_tensor: np.ndarray, _trace: bool = False):
    global _NC_CACHE
    x = np.ascontiguousarray(np.asarray(nd_tensor, dtype=np.float32)).reshape(
        32, 1024, 1024
    )
    pm = _build_pm()
    if _NC_CACHE is None:
        _NC_CACHE = build_nc()
    nc = _NC_CACHE

    in_maps = [
        {
            "x": np.ascontiguousarray(
                x[SLICES_PER_CORE * i : SLICES_PER_CORE * (i + 1)]
            ).reshape(ROWS, 1024),
            "pm": pm,
        }
        for i in range(N_CORES)
    ]
    res = run_bass_kernel_spmd(
        nc, in_maps, core_ids=list(range(N_CORES)), trace=_trace
    )
    out = np.stack([res.results[i]["y"] for i in range(N_CORES)])  # [8,1024,256]
    out = out.reshape(2, 16, 16, 16, 16, 16).astype(np.float32)
    if _trace:
        kernel.last_results = res
    return out
